# revision 68
# baseline (speedup 1.0000x reference)
"""Trainium2 Bass kernel for nn_EpochMixer: 2-layer post-norm transformer over
B*T independent 4-token epochs (CLS + 3 modalities), d_model=128, 8 heads,
ffn=512, run data-parallel over 8 NeuronCores (4096 epochs / 16384 tokens per
core).

Per-core design:
  - Residual stream feature-major [128=d_model partitions, 16384 tokens] in
    fp16 (fp16 keeps every DVE op in 2x mode at ~8x finer mantissa than
    bf16), SBUF-resident across both layers; token order is slot-major
    within each 128-token subchunk (slot*32+epoch), so CLS tokens are
    contiguous 32-partition / 32-column blocks.
  - QKV is emitted TOKEN-major by using the activation chunk as the matmul
    stationary operand, split into a double-buffered kv PSUM pool and a q
    pool so successive chunks' matmuls overlap the PSUM evacuations; the
    host pre-permutes Wqkv's V rows to (d-major,h-minor) so all three
    evacuations are contiguous copies spread over DVE+ACT.  Attention is
    lane-local: k/v partition rotations are 2 wrap-split 96/32-partition
    SBUF DMAs per delta (k on the SP HWDGE ring, v on SWDGE), scores = one
    fused [*,4delta,...] fp16 multiply + in-place binary-tree reduce,
    softmax exp on ACT with packed-pair denominator adds on DVE, AV = one
    fused multiply against the (d-major,h-minor) V + 3 adds.  All pools are
    layer-scoped so consecutive 2048-token groups pipeline.
  - LayerNorm (mean-free trick): Wo and W2 are column-centered on the host
    (C@W with C = I - 11^T/128), making every LN input exactly zero-mean
    except layer 0's first LN (explicit mu row).  LN = rstd scaling only:
    sumsq rows via ones-stationary matmuls staged into per-group SBUF rows
    (one DRAM DMA per 2048-token group), rstd = exp(-0.5*ln(ms+eps)) on
    ACT, and a partition-broadcast (0-stride) load + one 2x DVE multiply
    per group.  Attention-side LN runs per-group (Ln/Exp share the
    natural_log ACT table set with the softmax Exp, so no table thrash);
    FFN-side LN stays at phase end to avoid Gelu<->Ln set reloads.
  - Only CLS tokens feed the output, so the last layer runs phased: QKV for
    the whole stream into k/v stream tiles, quarter-stream dense-CLS packs
    (big block DMAs over 3 rings, no rotations), then dense attention and
    CLS-only Wo/residual/LN/FFN on the slot-major CLS columns.
  - Output is written feature-major fp16 ([128, 4096] strided CLS-column
    DMA, no on-device transpose); the host transposes and casts.
"""

import numpy as np
import ml_dtypes

B, T, M, D, H, L, F = 16, 2048, 3, 128, 8, 2, 512
S = M + 1
DH = D // H
NCORE = 8
EPC = B * T // NCORE          # 4096 epochs per core
NTOK = EPC * S                # 16384 tokens per core
CH = 512
NCH = NTOK // CH              # 32
NSUB = CH // 128              # 4
GRP = 4                       # chunks per attention group
NGRP = NCH // GRP
GCH = GRP * CH                # tokens per group (2048)
EPS = 1e-5

_BUILD_CACHE = {}


def _split_ctrl_waits(nc, mybir):
    """walrus here rejects >1 SyncWait per instruction: hoist extras onto
    single-wait NOPs inserted in front."""
    ctr = 0
    fn = nc.m.functions[0]
    for bb in fn.blocks:
        new_insts = []
        for ins in bb.instructions:
            si = getattr(ins, "sync_info", None)
            waits = list(si.on_wait) if si is not None and si.on_wait else []
            if len(waits) > 1:
                for w in waits[1:]:
                    ctr += 1
                    nop = mybir.InstNoOp(name=f"WSPLIT-{ctr}", ins=[], outs=[])
                    nop.engine = ins.engine
                    nop.sync_info = mybir.SyncInfo(on_wait=[w], on_update=[])
                    new_insts.append(nop)
                ins.sync_info = mybir.SyncInfo(
                    on_wait=waits[:1], on_update=list(si.on_update or []))
            new_insts.append(ins)
        bb.instructions = new_insts


def _build_program():
    import contextlib
    import concourse.bass as bass
    import concourse.tile as tile
    from concourse import mybir

    f32 = mybir.dt.float32
    f32r = mybir.dt.float32r
    bf16 = mybir.dt.float16  # fp16 everywhere a 2-byte dtype is wanted
    AF = mybir.ActivationFunctionType
    ALU = mybir.AluOpType
    AX = mybir.AxisListType
    SCALE = 1.0 / np.sqrt(DH)

    nc = bass.Bass()

    x0_d = nc.declare_dram_parameter("x0", [128, NTOK], bf16, isOutput=False)
    wq_d = [nc.declare_dram_parameter(f"wqkv{l}", [128, 3 * D], bf16, isOutput=False)
            for l in range(L)]
    wo_d = [nc.declare_dram_parameter(f"wo{l}", [128, D], bf16, isOutput=False)
            for l in range(L)]
    w1_d = [nc.declare_dram_parameter(f"w1{l}", [128, F], bf16, isOutput=False)
            for l in range(L)]
    w2_d = [nc.declare_dram_parameter(f"w2{l}", [128, F], bf16, isOutput=False)
            for l in range(L)]
    ones2_d = nc.declare_dram_parameter("ones2", [128, 2], bf16, isOutput=False)
    idb_d = nc.declare_dram_parameter("identb", [128, 128], bf16, isOutput=False)
    # feature-major f16 output; the host transposes/casts (free off-device)
    y_d = nc.declare_dram_parameter("y", [128, EPC], bf16, isOutput=True)

    scr_rs = nc.dram_tensor("scr_rs", [2 * L, NTOK], bf16)
    scr_mu = nc.dram_tensor("scr_mu", [NTOK], f32)
    scr_sq = nc.dram_tensor("scr_sq", [1, NTOK], f32)
    scr_sm = nc.dram_tensor("scr_sm", [1, NTOK], f32)

    def ln_rows(ln_idx, with_mu, scpool, ntok=NTOK, t0=0):
        """scr_sq[t0:t0+ntok] (sum of u^2 over features per token) -> fp16
        rstd row in DRAM scr_rs[ln_idx].  If with_mu (only LN0 of layer 0,
        whose input is not zero-mean), also writes -mu*rstd to scr_mu."""
        nr = ntok // 128
        tm = scpool.tile([128, nr], f32, tag="ln_tm")
        nc.sync.dma_start(
            out=tm,
            in_=scr_sq[0:1, t0:t0 + ntok].rearrange("p (q r) -> p q r",
                                                    q=128))
        rstd_b = scpool.tile([128, nr], bf16, tag="ln_rb")
        if not with_mu:
            lnv = scpool.tile([128, nr], f32, tag="ln_lnv")
            nc.scalar.activation(out=lnv, in_=tm, func=AF.Ln,
                                 bias=epst, scale=1.0 / 128.0)
            nc.scalar.activation(out=rstd_b, in_=lnv, func=AF.Exp, scale=-0.5)
        else:
            tms = scpool.tile([128, nr], f32, tag="ln_tms")
            nc.sync.dma_start(
                out=tms,
                in_=scr_sm[0:1, t0:t0 + ntok].rearrange("p (q r) -> p q r",
                                                        q=128))
            mu = scpool.tile([128, nr], f32, tag="ln_mu")
            nc.vector.tensor_scalar_mul(out=mu, in0=tms, scalar1=1.0 / 128.0)
            m2 = scpool.tile([128, nr], f32, tag="ln_m2")
            nc.vector.tensor_scalar_mul(out=m2, in0=tm, scalar1=1.0 / 128.0)
            musq = scpool.tile([128, nr], f32, tag="ln_musq")
            nc.vector.tensor_tensor(out=musq, in0=mu, in1=mu, op=ALU.mult)
            var = scpool.tile([128, nr], f32, tag="ln_var")
            nc.vector.tensor_tensor(out=var, in0=m2, in1=musq,
                                    op=ALU.subtract)
            lnv = scpool.tile([128, nr], f32, tag="ln_lnv")
            nc.scalar.activation(out=lnv, in_=var, func=AF.Ln, bias=epst)
            rstd = scpool.tile([128, nr], f32, tag="ln_rstd")
            nc.scalar.activation(out=rstd, in_=lnv, func=AF.Exp, scale=-0.5)
            nc.vector.tensor_copy(out=rstd_b, in_=rstd)
            nmur = scpool.tile([128, nr], f32, tag="ln_nmur")
            nc.vector.tensor_tensor(out=nmur, in0=mu, in1=rstd, op=ALU.mult)
            nmur2 = scpool.tile([128, nr], f32, tag="ln_nmur2")
            nc.vector.tensor_scalar_mul(out=nmur2, in0=nmur, scalar1=-1.0)
            nc.sync.dma_start(
                out=bass.AP(tensor=scr_mu, offset=t0,
                            ap=[[nr, 128], [1, nr]]),
                in_=nmur2)
        nc.sync.dma_start(
            out=bass.AP(tensor=scr_rs, offset=ln_idx * NTOK + t0,
                        ap=[[nr, 128], [1, nr]]),
            in_=rstd_b)

    def ln_apply(ln_idx, x, with_mu, cls_only=False, apool=None, g0=0,
                 g1=NGRP):
        """x = x * rstd_rep (+ (-mu*rstd)_rep for LN0), per 2048-token
        group via 0-stride partition-broadcast DMA loads.  cls_only: the
        rstd row is epoch-indexed and only CLS columns of x are updated
        (the rest of the stream is dead past this point)."""
        with contextlib.ExitStack() as actx_:
            if apool is None:
                apool = actx_.enter_context(
                    tc.tile_pool(name=f"lna{ln_idx}", bufs=2))
            if cls_only:
                for g in range(4):
                    e0 = g * (EPC // 4)
                    rep = apool.tile([128, EPC // 4], bf16, tag="ln_repc")
                    nc.sync.dma_start(
                        out=rep, in_=bass.AP(tensor=scr_rs,
                                             offset=ln_idx * NTOK + e0,
                                             ap=[[0, 128], [1, EPC // 4]]))
                    xv = x[:, 4 * e0:4 * (e0 + EPC // 4)].rearrange(
                        "p (b s e) -> p b s e", s=S, e=32)[:, :, 0, :]
                    nc.vector.tensor_tensor(out=xv, in0=xv, in1=rep,
                                            op=ALU.mult)
                return
            for g in range(g0, g1):
                t0 = g * GCH
                rep = apool.tile([128, GCH], bf16, tag="ln_rep")
                nc.gpsimd.dma_start(
                    out=rep, in_=bass.AP(tensor=scr_rs,
                                         offset=ln_idx * NTOK + t0,
                                         ap=[[0, 128], [1, GCH]]))
                nc.vector.tensor_tensor(out=x[:, t0:t0 + GCH],
                                        in0=x[:, t0:t0 + GCH], in1=rep,
                                        op=ALU.mult)
                if with_mu:
                    nc.gpsimd.dma_start(
                        out=x[:, t0:t0 + GCH],
                        in_=bass.AP(tensor=scr_mu, offset=t0,
                                    ap=[[0, 128], [1, GCH]]),
                        accum_op=ALU.add)

    with tile.TileContext(nc) as tc, contextlib.ExitStack() as top:
        consts = top.enter_context(tc.tile_pool(name="consts", bufs=1))
        resid = top.enter_context(tc.tile_pool(name="resid", bufs=1))

        epst = consts.tile([128, 1], f32, tag="epst")
        nc.vector.memset(epst, EPS)
        # wq0 first on the ACT ring: it gates the very first matmul
        wq_b = []
        for l in range(L):
            wqt = consts.tile([128, 3 * D], bf16, tag=f"wqb{l}")
            wq_b.append(wqt)
        nc.scalar.dma_start(out=wq_b[0], in_=wq_d[0][:, :])
        ones_b = consts.tile([128, 2], bf16)
        nc.scalar.dma_start(out=ones_b, in_=ones2_d[:, :])
        ident_b = consts.tile([128, 128], bf16)
        nc.scalar.dma_start(out=ident_b, in_=idb_d[:, :])

        wo_b, w1_b, w2_b = [], [], []
        for l in range(L):
            if l > 0:
                nc.scalar.dma_start(out=wq_b[l], in_=wq_d[l][:, :])
            t = consts.tile([128, D], bf16, tag=f"wob{l}")
            nc.scalar.dma_start(out=t, in_=wo_d[l][:, :])
            wo_b.append(t)
            t = consts.tile([128, F], bf16, tag=f"w1b{l}")
            nc.scalar.dma_start(out=t, in_=w1_d[l][:, :])
            w1_b.append(t)
            t = consts.tile([128, F], bf16, tag=f"w2b{l}")
            nc.scalar.dma_start(out=t, in_=w2_d[l][:, :])
            w2_b.append(t)

        # residual stream: per-group loads on alternating rings so the
        # first attention group starts after 1/8 of the stream has landed
        x = resid.tile([128, NTOK], bf16)
        for c in range(2 * NGRP):
            sl = slice(c * (GCH // 2), (c + 1) * (GCH // 2))
            eng = (nc.sync, nc.gpsimd, nc.scalar)[c % 3]
            eng.dma_start(out=x[:, sl], in_=x0_d[:, sl])

        for l in range(L):
            # =================== attention ===================
            GSUB = GRP * NSUB  # subchunks per group
            with contextlib.ExitStack() as actx:
                # Pools hoisted to layer scope so consecutive groups pipeline
                # (per-group pools would force WAR serialization on slot
                # reuse).  PSUM budget: pq 4 banks + pu 2 + misc 2 = 8.
                ap_ = actx.enter_context(
                    tc.tile_pool(name=f"qkv{l}", bufs=3 if l == L - 1 else 2))
                sctch = actx.enter_context(
                    tc.tile_pool(name=f"asc{l}", bufs=1))
                if l < L - 1:
                    # split kv/q PSUM pools: kv (2 banks) double-buffered so
                    # next chunk's matmuls don't wait on this chunk's
                    # copies; bank budget kv 4 + q 1 + pu 1 + misc 2 = 8
                    qkps = actx.enter_context(
                        tc.tile_pool(name=f"qkps{l}", bufs=2, space="PSUM"))
                    qkpsQ = actx.enter_context(
                        tc.tile_pool(name=f"qkpsq{l}", bufs=1, space="PSUM"))
                    ps_u = actx.enter_context(
                        tc.tile_pool(name=f"psu{l}", bufs=1, space="PSUM"))
                    ps_m = actx.enter_context(
                        tc.tile_pool(name=f"psmsc{l}", bufs=2, space="PSUM"))
                ev4 = actx.enter_context(
                    tc.tile_pool(name=f"ev4{l}", bufs=3))
                lnp = actx.enter_context(
                    tc.tile_pool(name=f"lnp{l}", bufs=2))
                last = (l == L - 1)
                if last:
                    # ---- last layer: phased whole-stream dense-CLS path.
                    # P1: qkv for all chunks into whole-stream k/v tiles;
                    # P2: half-stream dense packs (few big DMAs, 3 rings);
                    # P3: dense attention + Wo/stats per dense group.
                    dense = actx.enter_context(
                        tc.tile_pool(name=f"dense{l}", bufs=1))
                    qD_all = dense.tile([128, NCH, 128], bf16, tag="qDa")
                    kD_all = dense.tile([128, 4, NCH, 128], bf16, tag="kDa")
                    vD_all = dense.tile([128, 4, NCH, 128], bf16, tag="vDa")
                    with tc.tile_pool(name=f"kvall{l}", bufs=1) as kvp, \
                            tc.tile_pool(name=f"qkpsA{l}", bufs=1,
                                         space="PSUM") as qkps, \
                            tc.tile_pool(name=f"qkpsB{l}", bufs=1,
                                         space="PSUM") as qkpsB:
                        k_all = kvp.tile([128, NCH * NSUB, 128], bf16,
                                         tag="k_all")
                        v_all = kvp.tile([128, NCH * NSUB, 128], bf16,
                                         tag="v_all")
                        for g in range(NGRP):
                            q_grp = ap_.tile([128, GSUB, 128], bf16,
                                             tag="q_all")
                            for ci in range(GRP):
                                c = g * GRP + ci
                                # alternate two 4-bank pools: full
                                # double-buffering of the qkv PSUM slot
                                pq = (qkps, qkpsB)[c % 2].tile(
                                    [128, NSUB, 512], f32, tag="pqkv")
                                for s4 in range(NSUB):
                                    t0 = c * CH + s4 * 128
                                    nc.tensor.matmul(pq[:, s4, 0:3 * D],
                                                     x[:, t0:t0 + 128],
                                                     wq_b[l],
                                                     start=True, stop=True)
                                csl = slice(ci * NSUB, (ci + 1) * NSUB)
                                gsl = slice(c * NSUB, (c + 1) * NSUB)
                                # balance 3 evacuation copies over the two
                                # engines by alternating q's engine
                                if c % 2 == 0:
                                    nc.vector.tensor_copy(
                                        out=q_grp[:, csl, :],
                                        in_=pq[:, :, 0:128])
                                else:
                                    nc.scalar.activation(
                                        out=q_grp[:, csl, :],
                                        in_=pq[:, :, 0:128], func=AF.Copy)
                                nc.vector.tensor_copy(out=k_all[:, gsl, :],
                                                      in_=pq[:, :, 128:256])
                                nc.scalar.activation(out=v_all[:, gsl, :],
                                                     in_=pq[:, :, 256:384],
                                                     func=AF.Copy)
                            for m in range(4):
                                dp = slice(m * 32, (m + 1) * 32)
                                nc.sync.dma_start(
                                    out=qD_all[dp, g * GRP:(g + 1) * GRP, :],
                                    in_=q_grp[0:32].rearrange(
                                        "p (k m) f -> p m k f",
                                        m=NSUB)[:, m])
                            if g % 2 == 1:
                                # quarter-stream dense k/v pack waves: fire
                                # after every 2 groups for finer overlap
                                hsl = slice((g - 1) * GRP, (g + 1) * GRP)
                                for m in range(4):
                                    dp = slice(m * 32, (m + 1) * 32)
                                    for j in range(4):
                                        jp = slice(j * 32, (j + 1) * 32)
                                        # spread packs over 3 DMA rings
                                        keng = (nc.sync, nc.scalar)[j % 2]
                                        keng.dma_start(
                                            out=kD_all[dp, j, hsl],
                                            in_=k_all[jp].rearrange(
                                                "p (C m) f -> p m C f",
                                                m=NSUB)[:, m, hsl])
                                        nc.gpsimd.dma_start(
                                            out=vD_all[dp, j, hsl],
                                            in_=v_all[jp].rearrange(
                                                "p (C m) f -> p m C f",
                                                m=NSUB)[:, m, hsl])
                    ps_u = actx.enter_context(
                        tc.tile_pool(name=f"psu{l}", bufs=2, space="PSUM"))
                    ps_m = actx.enter_context(
                        tc.tile_pool(name=f"psmsc{l}", bufs=2, space="PSUM"))
                    for g in range(NGRP):
                        c0 = g * GRP
                        dsl = slice(c0, c0 + GRP)
                        qD = qD_all[:, dsl]
                        kD = kD_all[:, :, dsl]
                        vD = vD_all[:, :, dsl]
                        aoD = ap_.tile([128, GRP, 128], bf16, tag="aoD")
                        sstD_row = lnp.tile([1, GRP * 128], f32,
                                            tag="sstrow")
                        # ---- A3 (dense): scores / softmax / AV ----
                        ppD = sctch.tile([128, 4, GRP, H, DH], bf16,
                                         tag="ppD")
                        nc.vector.tensor_tensor(
                            out=ppD,
                            in0=qD.rearrange("p k (h d) -> p k h d",
                                             h=H).unsqueeze(1)
                                .broadcast_to((128, 4, GRP, H, DH)),
                            in1=kD.rearrange("p j k (h d) -> p j k h d",
                                             h=H),
                            op=ALU.mult)
                        sttD = sctch.tile([128, 4, GRP, H], bf16,
                                          tag="sttD")
                        for w in (8, 4, 2):
                            nc.vector.tensor_tensor(
                                out=ppD[:, :, :, :, 0:w],
                                in0=ppD[:, :, :, :, 0:w],
                                in1=ppD[:, :, :, :, w:2 * w], op=ALU.add)
                        nc.vector.tensor_tensor(
                            out=sttD, in0=ppD[:, :, :, :, 0],
                            in1=ppD[:, :, :, :, 1], op=ALU.add)
                        eeD = sctch.tile([128, 4, GRP, H], bf16,
                                         tag="eeD")
                        nc.scalar.activation(out=eeD, in_=sttD,
                                             func=AF.Exp, scale=SCALE)
                        # den via two packed 2x adds (tensor_reduce over the
                        # transposed view runs 1x with strided access)
                        e2D = sctch.tile([128, 2, GRP, H], bf16, tag="e2D")
                        nc.vector.tensor_tensor(out=e2D, in0=eeD[:, 0:2],
                                                in1=eeD[:, 2:4], op=ALU.add)
                        denD = sctch.tile([128, GRP, H], bf16, tag="denD")
                        nc.vector.tensor_tensor(out=denD, in0=e2D[:, 0],
                                                in1=e2D[:, 1], op=ALU.add)
                        rcpD = sctch.tile([128, GRP, H], bf16,
                                          tag="rcpD")
                        with nc.allow_low_precision("softmax rcp"):
                            nc.vector.reciprocal(out=rcpD, in_=denD)
                        aaD = sctch.tile([128, 4, GRP, H], bf16,
                                         tag="aaD")
                        nc.vector.tensor_tensor(
                            out=aaD, in0=eeD,
                            in1=rcpD.unsqueeze(1).broadcast_to(
                                (128, 4, GRP, H)),
                            op=ALU.mult)
                        tD = sctch.tile([128, 4, GRP, 128], bf16,
                                        tag="tD")
                        nc.vector.tensor_tensor(
                            out=tD, in0=vD,
                            in1=aaD.unsqueeze(3).broadcast_to(
                                (128, 4, GRP, DH, H)),
                            op=ALU.mult)
                        nc.vector.tensor_tensor(out=tD[:, 0:2],
                                                in0=tD[:, 0:2],
                                                in1=tD[:, 2:4], op=ALU.add)
                        nc.vector.tensor_tensor(out=aoD, in0=tD[:, 0],
                                                in1=tD[:, 1], op=ALU.add)
                        # ---- A4 (dense): Wo + residual + stats, CLS only --
                        for ci in range(GRP):
                            c = c0 + ci
                            t0 = c * CH
                            ao_fm = ev4.tile([128, 128], bf16, tag="ao_fm")
                            xv = x[:, t0:t0 + CH].rearrange(
                                "p (s4 s e) -> p s4 s e", s=S, e=32
                                )[:, :, 0, :]
                            ptc = ps_m.tile([128, 128], bf16, tag="misc")
                            nc.tensor.matmul(ptc, aoD[:, ci, :],
                                             ident_b, is_transpose=True)
                            nc.vector.tensor_copy(out=ao_fm, in_=ptc)
                            pu = ps_u.tile([128, 128], f32, tag="pu")
                            nc.tensor.matmul(pu, wo_b[l], ao_fm,
                                             start=True, stop=False)
                            nc.tensor.matmul(pu, ident_b, xv,
                                             start=False, stop=True)
                            nc.scalar.activation(out=xv, in_=pu,
                                                 func=AF.Copy)
                            sq = ev4.tile([128, 128], bf16, tag="sq1")
                            nc.vector.tensor_tensor(out=sq, in0=xv, in1=xv,
                                                    op=ALU.mult)
                            psq = ps_m.tile([1, 128], f32, tag="misc")
                            nc.tensor.matmul(psq, ones_b[:, 0:1], sq,
                                             start=True, stop=True)
                            nc.scalar.activation(
                                out=sstD_row[0:1, ci * 128:(ci + 1) * 128],
                                in_=psq, func=AF.Copy)
                        nc.sync.dma_start(
                            out=scr_sq[0:1, c0 * 128:(c0 + GRP) * 128],
                            in_=sstD_row)
                    with contextlib.ExitStack() as lctx:
                        lnsc = lctx.enter_context(tc.tile_pool(
                            name=f"lnsc_a{l}", bufs=1))
                        ln_rows(2 * l + 0, False, lnsc, ntok=EPC)
                        ln_apply(2 * l + 0, x, with_mu=False, cls_only=True)
                for g in range(NGRP if not last else 0):
                    c0 = g * GRP
                    if True:
                        q_all = ap_.tile([128, GSUB, 128], bf16, tag="q_all")
                        kS4 = ap_.tile([128, 4, GSUB, 128], bf16,
                                       tag="kS4")
                        vS4 = ap_.tile([128, 4, GSUB, 128], bf16,
                                       tag="vS4")
                        ao_tm = ap_.tile([128, GSUB, 128], bf16,
                                         tag="ao_tm")

                        # ---- A1: token-major qkv (kv and q in separate
                        # PSUM pools; kv double-buffered) ----
                        for ci in range(GRP):
                            c = c0 + ci
                            pkv = qkps.tile([128, NSUB, 256], f32,
                                            tag="pqkv")
                            pqq = qkpsQ.tile([128, NSUB, 128], f32,
                                             tag="pqq")
                            for s4 in range(NSUB):
                                t0 = c * CH + s4 * 128
                                nc.tensor.matmul(pkv[:, s4, :],
                                                 x[:, t0:t0 + 128],
                                                 wq_b[l][:, 128:384],
                                                 start=True, stop=True)
                                nc.tensor.matmul(pqq[:, s4, :],
                                                 x[:, t0:t0 + 128],
                                                 wq_b[l][:, 0:128],
                                                 start=True, stop=True)
                            csl = slice(ci * NSUB, (ci + 1) * NSUB)
                            # q/v evacuation on ACT (DVE is the attention
                            # bottleneck; ACT has slack here), k on DVE
                            nc.scalar.activation(out=q_all[:, csl, :],
                                                 in_=pqq,
                                                 func=AF.Copy)
                            # k on DVE for the first two groups (DVE has no
                            # attention work yet and ACT gates the ramp),
                            # on ACT afterwards (DVE becomes the gate)
                            if g < 2:
                                nc.vector.tensor_copy(
                                    out=kS4[:, 0, csl, :],
                                    in_=pkv[:, :, 0:128])
                            else:
                                nc.scalar.activation(
                                    out=kS4[:, 0, csl, :],
                                    in_=pkv[:, :, 0:128], func=AF.Copy)
                            # host pre-permutes Wqkv's V rows to
                            # (d-major, h-minor), so this is contiguous
                            nc.scalar.activation(out=vS4[:, 0, csl, :],
                                                 in_=pkv[:, :, 128:256],
                                                 func=AF.Copy)

                        if True:
                            # ---- A2: partition rotations (slot-major:
                            # slots are contiguous 32-partition blocks).
                            # Batched into 2 contiguous-partition-range DMAs
                            # per (delta, tensor) (wrap split), k on the SP
                            # HWDGE ring and v on the SWDGE ring so the two
                            # streams drain in parallel. ----
                            for dlt in (1, 2, 3):
                                cut = (S - dlt) * 32
                                for srct, eng in ((kS4, nc.sync),
                                                  (vS4, nc.gpsimd)):
                                    eng.dma_start(
                                        out=srct[0:cut, dlt],
                                        in_=srct[dlt * 32:128, 0])
                                    eng.dma_start(
                                        out=srct[cut:128, dlt],
                                        in_=srct[0:dlt * 32, 0])

                        # ---- A3: scores / softmax / AV ----
                        if True:
                            pp4 = sctch.tile([128, 4, GSUB, H, DH], bf16,
                                             tag="pp4")
                            # per-delta multiplies: each depends only on its
                            # own rotation DMA (delta 0 on none), so the
                            # multiply pipelines against the rotations
                            qv = q_all.rearrange("p c (h d) -> p c h d", h=H)
                            for dlt in range(4):
                                nc.vector.tensor_tensor(
                                    out=pp4[:, dlt],
                                    in0=qv,
                                    in1=kS4[:, dlt].rearrange(
                                        "p c (h d) -> p c h d", h=H),
                                    op=ALU.mult)
                            stt4 = sctch.tile([128, 4, GSUB, H], bf16,
                                              tag="stt4")
                            for w in (8, 4, 2):
                                nc.vector.tensor_tensor(
                                    out=pp4[:, :, :, :, 0:w],
                                    in0=pp4[:, :, :, :, 0:w],
                                    in1=pp4[:, :, :, :, w:2 * w], op=ALU.add)
                            nc.vector.tensor_tensor(
                                out=stt4, in0=pp4[:, :, :, :, 0],
                                in1=pp4[:, :, :, :, 1], op=ALU.add)
                            ee = sctch.tile([128, 4, GSUB, H], bf16, tag="ee")
                            nc.scalar.activation(out=ee, in_=stt4, func=AF.Exp,
                                                 scale=SCALE)
                            # den via two packed 2x adds (tensor_reduce over
                            # the transposed view runs 1x, strided)
                            e2 = sctch.tile([128, 2, GSUB, H], bf16,
                                            tag="e2")
                            nc.vector.tensor_tensor(out=e2, in0=ee[:, 0:2],
                                                    in1=ee[:, 2:4],
                                                    op=ALU.add)
                            den = sctch.tile([128, GSUB, H], bf16, tag="den")
                            nc.vector.tensor_tensor(out=den, in0=e2[:, 0],
                                                    in1=e2[:, 1], op=ALU.add)
                            rcp = sctch.tile([128, GSUB, H], bf16, tag="rcp")
                            with nc.allow_low_precision("softmax rcp in bf16"):
                                nc.vector.reciprocal(out=rcp, in_=den)
                            aa = sctch.tile([128, 4, GSUB, H], bf16, tag="aa")
                            nc.vector.tensor_tensor(
                                out=aa, in0=ee,
                                in1=rcp.unsqueeze(1).broadcast_to(
                                    (128, 4, GSUB, H)),
                                op=ALU.mult)
                            # t4[d,h] layout matches vS4; aa broadcast over DH
                            # lands on a packed last (h) axis so the mult runs 2x
                            t4 = sctch.tile([128, 4, GSUB, 128], bf16, tag="t4")
                            # per-delta AV multiplies: each waits only on
                            # its own v-rotation (delta 0 on none)
                            for dlt in range(4):
                                nc.vector.tensor_tensor(
                                    out=t4[:, dlt], in0=vS4[:, dlt],
                                    in1=aa[:, dlt].unsqueeze(2).broadcast_to(
                                        (128, GSUB, DH, H)),
                                    op=ALU.mult)
                            # pair-tree: one in-place packed add + one
                            # final add (2 ops instead of 3 serial adds)
                            nc.vector.tensor_tensor(out=t4[:, 0:2],
                                                    in0=t4[:, 0:2],
                                                    in1=t4[:, 2:4],
                                                    op=ALU.add)
                            nc.vector.tensor_tensor(out=ao_tm, in0=t4[:, 0],
                                                    in1=t4[:, 1], op=ALU.add)

                        # ---- A4: Wo + residual (u -> x in place) + stats ----
                        W = 512
                        sst_row = lnp.tile([1, GCH], f32, tag="sstrow")
                        if l == 0:
                            smt_row = lnp.tile([1, GCH], f32, tag="smtrow")
                        for ci in range(GRP):
                            c = c0 + ci
                            t0 = c * CH
                            ao_fm = ev4.tile([128, W], bf16, tag="ao_fm")
                            if True:
                                xv = x[:, t0:t0 + CH]
                                pt4 = ps_m.tile([128, NSUB, 128], bf16,
                                                tag="misc")
                                for s4 in range(NSUB):
                                    nc.tensor.matmul(
                                        pt4[:, s4],
                                        ao_tm[:, ci * NSUB + s4, :],
                                        ident_b, is_transpose=True)
                                nc.vector.tensor_copy(
                                    out=ao_fm.rearrange(
                                        "p (s4 w) -> p s4 w", s4=NSUB),
                                    in_=pt4)
                            pu = ps_u.tile([128, W], f32, tag="pu")
                            nc.tensor.matmul(pu, wo_b[l], ao_fm,
                                             start=True, stop=False)
                            nc.tensor.matmul(pu, ident_b, xv,
                                             start=False, stop=True)
                            # u1 -> x in place on ACT; square as a 2x DVE
                            # TT on the f16 result (cheaper than 1x PSUM)
                            nc.scalar.activation(out=xv, in_=pu,
                                                 func=AF.Copy)
                            sq = ev4.tile([128, W], bf16, tag="sq1")
                            nc.vector.tensor_tensor(out=sq, in0=xv, in1=xv,
                                                    op=ALU.mult)
                            psq = ps_m.tile([1, W], f32, tag="misc")
                            nc.tensor.matmul(psq, ones_b[:, 0:1], sq,
                                             start=True, stop=True)
                            # stats staged into per-group SBUF rows; one
                            # DMA per group instead of one per chunk
                            nc.scalar.activation(
                                out=sst_row[0:1, ci * W:(ci + 1) * W],
                                in_=psq, func=AF.Copy)
                            if l == 0:
                                psm = ps_m.tile([1, 512], f32, tag="misc")
                                nc.tensor.matmul(psm, ones_b[:, 0:1],
                                                 x[:, t0:t0 + CH],
                                                 start=True, stop=True)
                                nc.scalar.activation(
                                    out=smt_row[0:1, ci * CH:(ci + 1) * CH],
                                    in_=psm, func=AF.Copy)

                        nc.sync.dma_start(
                            out=scr_sq[0:1, g * GCH:(g + 1) * GCH],
                            in_=sst_row)
                        if l == 0:
                            nc.sync.dma_start(
                                out=scr_sm[0:1, g * GCH:(g + 1) * GCH],
                                in_=smt_row)
                        # per-group LN1: Ln/Exp share the natural_log table
                        # set with the softmax Exp, so no ACT set thrash
                        # (unlike Gelu-interleaved LN in the FFN loop)
                        ln_rows(2 * l + 0, l == 0, lnp, ntok=GCH,
                                t0=g * GCH)
                        ln_apply(2 * l + 0, x, with_mu=(l == 0),
                                 apool=lnp, g0=g, g1=g + 1)

            # =================== FFN ===================
            with contextlib.ExitStack() as fctx:
                fp = fctx.enter_context(tc.tile_pool(name=f"ffn{l}", bufs=1))
                fsc = fctx.enter_context(tc.tile_pool(name=f"fsc{l}", bufs=3))
                last = (l == L - 1)
                nch_f = (EPC // CH) if last else NCH
                with contextlib.ExitStack() as floop:
                    fps = floop.enter_context(tc.tile_pool(
                        name=f"fps{l}", bufs=2, space="PSUM"))
                    fps2 = floop.enter_context(tc.tile_pool(
                        name=f"fps2{l}", bufs=2, space="PSUM"))
                    def fxv(c):
                        t0 = c * CH
                        if last:
                            # CLS tokens only: slot-major 32-wide blocks
                            return x[:, 4 * t0:4 * (t0 + CH)].rearrange(
                                "p (b s e) -> p b s e", s=S, e=32)[:, :, 0, :]
                        return x[:, t0:t0 + CH]

                    # chunk-paired: consecutive matmuls for the two chunks
                    # share the stationary weight block (halves LDWEIGHTS)
                    for cp in range(nch_f // 2):
                        ca, cb = 2 * cp, 2 * cp + 1
                        xva, xvb = fxv(ca), fxv(cb)
                        hha = fsc.tile([128, 4, 512], bf16, tag="hh")
                        hhb = fsc.tile([128, 4, 512], bf16, tag="hhb")
                        for half in range(2):
                            pha = fps.tile([128, 2, 512], f32, tag="ph")
                            phb = fps.tile([128, 2, 512], f32, tag="ph")
                            for jj in range(2):
                                j = half * 2 + jj
                                w1j = w1_b[l][:, j * 128:(j + 1) * 128]
                                nc.tensor.matmul(pha[:, jj, :], w1j, xva,
                                                 start=True, stop=True)
                                nc.tensor.matmul(phb[:, jj, :], w1j, xvb,
                                                 start=True, stop=True)
                            nc.scalar.activation(
                                out=hha[:, half * 2:half * 2 + 2, :],
                                in_=pha, func=AF.Gelu)
                            nc.scalar.activation(
                                out=hhb[:, half * 2:half * 2 + 2, :],
                                in_=phb, func=AF.Gelu)
                        pfa = fps2.tile([128, 512], f32, tag="pf")
                        pfb = fps2.tile([128, 512], f32, tag="pf")
                        for j in range(4):
                            w2j = w2_b[l][:, j * 128:(j + 1) * 128]
                            nc.tensor.matmul(pfa, w2j, hha[:, j, :],
                                             start=(j == 0), stop=(j == 3))
                            nc.tensor.matmul(pfb, w2j, hhb[:, j, :],
                                             start=(j == 0), stop=(j == 3))
                        for c, xv, pf in ((ca, xva, pfa), (cb, xvb, pfb)):
                            # u2 = x2 + ff -> x in place
                            nc.vector.tensor_tensor(out=xv, in0=xv,
                                                    in1=pf, op=ALU.add)
                            sq = fsc.tile([128, 512], bf16, tag="sq2")
                            nc.vector.tensor_tensor(out=sq, in0=xv, in1=xv,
                                                    op=ALU.mult)
                            psq = fps2.tile([1, 512], f32, tag="psqf")
                            nc.tensor.matmul(psq, ones_b[:, 0:1], sq,
                                             start=True, stop=True)
                            if c % GRP == 0:
                                fsst_row = fsc.tile([1, GRP * CH], f32,
                                                    tag="sstf")
                            nc.vector.tensor_copy(
                                out=fsst_row[0:1, (c % GRP) * CH:
                                             (c % GRP + 1) * CH],
                                in_=psq)
                            if c % GRP == GRP - 1:
                                nc.gpsimd.dma_start(
                                    out=scr_sq[0:1, (c - GRP + 1) * CH:
                                               (c + 1) * CH],
                                    in_=fsst_row)
                with contextlib.ExitStack() as lctx:
                    lnsc = lctx.enter_context(tc.tile_pool(
                        name=f"lnsc_f{l}", bufs=1))
                    ln_rows(2 * l + 1, False, lnsc,
                            ntok=(EPC if last else NTOK))
                    ln_apply(2 * l + 1, x, with_mu=False, cls_only=last)

        # =================== CLS extraction ===================
        # Straight strided DMA of the CLS columns; no transpose on-device.
        # 8 slices over 3 rings so each LN-apply group's columns stream out
        # while the next group's apply runs.
        xc = x.rearrange("p (b s e) -> p b s e", s=S, e=32)[:, :, 0, :]
        for g in range(8):
            sl = slice(g * (EPC // 8), (g + 1) * (EPC // 8))
            eng = (nc.sync, nc.gpsimd, nc.scalar)[g % 3]
            eng.dma_start(out=y_d[:, sl], in_=xc[:, sl.start // 32:
                                                 sl.stop // 32, :])

    _split_ctrl_waits(nc, mybir)
    return nc


def _get_program():
    if "nc" not in _BUILD_CACHE:
        _BUILD_CACHE["nc"] = _build_program()
    return _BUILD_CACHE["nc"]


def build_in_maps(z0, z1, z2, cls_token, Wqkv, Wo, W1, W2):
    """Host-side input prep shared by kernel() and test.py's profile path.
    Wo and W2 are column-centered (C@W with C = I - 11^T/128): their outputs
    then have exactly zero feature-mean, which together with LN's own
    centering makes every LN after the first mean-free."""
    bf = np.float16
    cls = np.asarray(cls_token, np.float32).reshape(D)
    zs = [np.asarray(z, np.float32).reshape(B * T, D) for z in (z0, z1, z2)]
    base = {
        "identb": np.eye(128, dtype=bf),
        "ones2": np.stack([np.ones(128, bf), np.zeros(128, bf)], 1),
    }
    # attention output features are stored (d-major, h-minor): f' = d*8+h
    dh_perm = np.arange(128).reshape(H, DH).T.reshape(-1)  # f'(d,h) -> h*16+d
    for l in range(L):
        Wo_c = Wo[l] - Wo[l].mean(axis=0, keepdims=True)
        W2_c = W2[l] - W2[l].mean(axis=0, keepdims=True)
        wq_t = np.ascontiguousarray(Wqkv[l].T)
        # V output features pre-permuted to (d-major, h-minor) so the
        # kernel's PSUM->SBUF V copy is contiguous (vS4 bytes unchanged).
        wq_t[:, 256:384] = wq_t[:, 256:384][:, dh_perm]
        base[f"wqkv{l}"] = wq_t.astype(bf)
        base[f"wo{l}"] = np.ascontiguousarray(Wo_c.T[dh_perm]).astype(bf)
        base[f"w1{l}"] = np.ascontiguousarray(W1[l].T).astype(bf)
        base[f"w2{l}"] = np.ascontiguousarray(
            W2_c.T.reshape(4, 128, 128).transpose(1, 0, 2).reshape(128, 512)
        ).astype(bf)

    in_maps = []
    for c in range(NCORE):
        e0, e1 = c * EPC, (c + 1) * EPC
        x0 = np.empty((128, NTOK), np.float32)
        xv = x0.reshape(128, EPC // 32, S, 32)
        xv[:, :, 0, :] = cls[:, None, None]
        for m in range(M):
            xv[:, :, 1 + m, :] = zs[m][e0:e1].T.reshape(128, EPC // 32, 32)
        in_maps.append({**base, "x0": x0.astype(bf)})
    return in_maps


# ==========================================================================
def kernel(z0, z1, z2, cls_token, Wqkv, bqkv, Wo, bo, W1, b1, W2, b2,
           ln1_g, ln1_b, ln2_g, ln2_b):
    import concourse.bass_utils as bass_utils

    z0 = np.asarray(z0, np.float32)
    z1 = np.asarray(z1, np.float32)
    z2 = np.asarray(z2, np.float32)
    cls = np.asarray(cls_token, np.float32).reshape(D)
    Wqkv = np.asarray(Wqkv, np.float32)
    Wo = np.asarray(Wo, np.float32)
    W1 = np.asarray(W1, np.float32)
    W2 = np.asarray(W2, np.float32)

    # fast path exploits the module's zero biases / unit gains
    for tns, want in ((bqkv, 0), (bo, 0), (b1, 0), (b2, 0),
                      (ln1_b, 0), (ln2_b, 0), (ln1_g, 1), (ln2_g, 1)):
        if not np.allclose(np.asarray(tns, np.float32), want, atol=1e-6):
            return _numpy_fallback(
                z0, z1, z2, cls, Wqkv, np.asarray(bqkv, np.float32),
                Wo, np.asarray(bo, np.float32), W1, np.asarray(b1, np.float32),
                W2, np.asarray(b2, np.float32),
                np.asarray(ln1_g, np.float32), np.asarray(ln1_b, np.float32),
                np.asarray(ln2_g, np.float32), np.asarray(ln2_b, np.float32))

    nc = _get_program()
    in_maps = build_in_maps(z0, z1, z2, cls, Wqkv, Wo, W1, W2)
    res = bass_utils.run_bass_kernel_spmd(nc, in_maps, list(range(NCORE)))
    out = np.empty((B * T, D), np.float32)
    for c in range(NCORE):
        out[c * EPC:(c + 1) * EPC] = res.results[c]["y"].T
    return out.reshape(B, T, D)


def _numpy_fallback(z0, z1, z2, cls, Wqkv, bqkv, Wo, bo, W1, b1, W2, b2,
                    g1, be1, g2, be2):
    from scipy.special import erf
    N = B * T
    z = np.stack([z0.reshape(N, D), z1.reshape(N, D), z2.reshape(N, D)], 1)
    xx = np.concatenate([np.broadcast_to(cls, (N, 1, D)), z], 1)

    def ln(v, g, b):
        mu = v.mean(-1, keepdims=True)
        var = ((v - mu) ** 2).mean(-1, keepdims=True)
        return (v - mu) / np.sqrt(var + EPS) * g + b

    for l in range(L):
        qkv = xx @ Wqkv[l].T + bqkv[l]
        q, k, v = np.split(qkv, 3, -1)
        q = q.reshape(N, S, H, DH)
        k = k.reshape(N, S, H, DH)
        v = v.reshape(N, S, H, DH)
        s = np.einsum('nihd,njhd->nhij', q, k) / np.sqrt(DH)
        e = np.exp(s - s.max(-1, keepdims=True))
        a = e / e.sum(-1, keepdims=True)
        o = np.einsum('nhij,njhd->nihd', a, v).reshape(N, S, D)
        xx = ln(xx + (o @ Wo[l].T + bo[l]), g1[l], be1[l])
        h = xx @ W1[l].T + b1[l]
        h = 0.5 * h * (1 + erf(h / np.sqrt(2)))
        xx = ln(xx + (h @ W2[l].T + b2[l]), g2[l], be2[l])
    return xx[:, 0, :].reshape(B, T, D)

